# revision 9
# baseline (speedup 1.0000x reference)
"""Trainium2 Bass kernel for an LSTM caption decoder (teacher-forced).

Strategy (8 NeuronCores, SPMD):
  - The LSTM recurrence (63 sequential steps) is replicated on all cores in
    fp32; the big fc projection [H=512 -> V=32000] is vocab-sharded (4000
    per core).
  - On steps whose argmax feeds the next token (use_tf[t] == False), each
    core computes its local top-1 over its vocab shard, cores AllGather the
    (value, index) pairs, and every core selects the global argmax token.
  - The full logits output [B, S-1, V] is recomputed after the recurrence as
    one large batched matmul in float32r (reduced-precision fp32, 4x faster,
    ~1.4e-4 rel err), vocab-sharded, and DMA'd out.
  - The teacher-forcing pattern (use_tf) is baked into the generated program
    at build time; captions/weights remain runtime data.
"""

import os
import numpy as np

V, E, H = 32000, 256, 512
B, S = 64, 64
NCORES = 8
VS = V // NCORES          # 4000 vocab per core
NCHUNK = 500              # fc free-dim chunk (one PSUM bank)
NCH = VS // NCHUNK        # 8 chunks


def _build_nc(use_tf):
    import concourse.bass as bass
    import concourse.bacc as bacc
    import concourse.mybir as mybir
    from concourse.tile import TileContext
    from contextlib import ExitStack

    F32 = mybir.dt.float32
    F32R = mybir.dt.float32r
    I32 = mybir.dt.int32
    U32 = mybir.dt.uint32
    AF = mybir.ActivationFunctionType
    OP = mybir.AluOpType
    AX = mybir.AxisListType

    S_ = len(use_tf)
    NT = S_ - 1               # number of recurrence steps (t = 1..S_-1)
    KH = H // 128             # 4 k-tiles over hidden
    KE = E // 128             # 2 k-tiles over embed
    ROWS = NT * B             # stored h rows (h_1..h_{NT})
    G4 = 4 * H                # 2048 gate width

    nc = bacc.Bacc("TRN2", target_bir_lowering=False, debug=False,
                   num_devices=NCORES)

    encT = nc.dram_tensor("encT", [H, B], F32, kind="ExternalInput")
    caps = nc.dram_tensor("caps", [B, S_], I32, kind="ExternalInput")
    Wg = nc.dram_tensor("Wg", [E + H, G4], F32, kind="ExternalInput")
    bg = nc.dram_tensor("bg", [1, G4], F32, kind="ExternalInput")
    fcw = nc.dram_tensor("fcw", [H, VS], F32, kind="ExternalInput")
    fcb = nc.dram_tensor("fcb", [1, VS], F32, kind="ExternalInput")
    emb = nc.dram_tensor("emb", [V, E], F32, kind="ExternalInput")
    coreoff = nc.dram_tensor("coreoff", [1, 1], F32, kind="ExternalInput")
    ident = nc.dram_tensor("ident", [128, 128], F32, kind="ExternalInput")
    cbase = nc.dram_tensor("cbase", [1, 8 * NCH], F32, kind="ExternalInput")
    ar64 = nc.dram_tensor("ar64", [1, 8 * NCH], F32, kind="ExternalInput")
    ar8 = nc.dram_tensor("ar8", [1, 8], F32, kind="ExternalInput")

    out = nc.dram_tensor("out", [B, NT, VS], F32, kind="ExternalOutput")
    toks = nc.dram_tensor("toks", [B, NT], I32, kind="ExternalOutput")
    hTd = nc.dram_tensor("hTd", [H, ROWS], F32)   # internal: transposed h's

    with TileContext(nc) as tc, ExitStack() as es:
        cp = es.enter_context(tc.tile_pool(name="const", bufs=1))
        dp = es.enter_context(tc.tile_pool(name="dram", bufs=2, space="DRAM"))
        gp = es.enter_context(tc.tile_pool(name="gates", bufs=1))
        ep = es.enter_context(tc.tile_pool(name="elem", bufs=1))
        wp = es.enter_context(tc.tile_pool(name="work", bufs=2))
        sp = es.enter_context(tc.tile_pool(name="small", bufs=2))
        psA = es.enter_context(tc.tile_pool(name="psA", bufs=1, space="PSUM"))
        psB = es.enter_context(tc.tile_pool(name="psB", bufs=3, space="PSUM"))
        psT = es.enter_context(tc.tile_pool(name="psT", bufs=1, space="PSUM"))

        # ---------------- persistent tiles ----------------
        hT_win = cp.tile([128, KH * 2 * B], F32)   # rolling 2-step window of h^T
        fcw_sb = cp.tile([128, KH * VS], F32)
        bg_sb = cp.tile([B, G4], F32)
        fcb_sb = cp.tile([128, VS], F32)
        id_sb = cp.tile([128, 128], F32)
        caps_sb = cp.tile([B, S_], I32)
        co_sb = cp.tile([B, 1], F32)
        cb_sb = cp.tile([B, 8 * NCH], F32)
        a64_sb = cp.tile([B, 8 * NCH], F32)
        a8_sb = cp.tile([B, 8], F32)
        c_sb = cp.tile([B, H], F32)
        tokdbg = cp.tile([B, NT], I32)

        def win(k, t):  # hT window column slice for step t, k-tile k
            base = k * 2 * B + (t % 2) * B
            return hT_win[:, base:base + B]

        for k in range(KH):
            nc.sync.dma_start(win(k, 0), encT[k * 128:(k + 1) * 128, :])
            nc.sync.dma_start(fcw_sb[:, k * VS:(k + 1) * VS],
                              fcw[k * 128:(k + 1) * 128, :])
        nc.sync.dma_start(bg_sb[:, :], bg[:, :].to_broadcast((B, G4)))
        nc.sync.dma_start(fcb_sb[:, :], fcb[:, :].to_broadcast((128, VS)))
        nc.sync.dma_start(id_sb[:, :], ident[:, :])
        nc.sync.dma_start(caps_sb[:, :], caps[:, :])
        nc.sync.dma_start(co_sb[:, :], coreoff[:, :].to_broadcast((B, 1)))
        nc.sync.dma_start(cb_sb[:, :], cbase[:, :].to_broadcast((B, 8 * NCH)))
        nc.sync.dma_start(a64_sb[:, :], ar64[:, :].to_broadcast((B, 8 * NCH)))
        nc.sync.dma_start(a8_sb[:, :], ar8[:, :].to_broadcast((B, 8)))
        nc.vector.memset(c_sb[:, :], 0.0)
        nc.vector.memset(tokdbg[:, :], 0)

        wgp_cm = tc.tile_pool(name="wg", bufs=1)
        wgp = wgp_cm.__enter__()
        wg_sb = wgp.tile([128, (KE + KH) * G4], F32)
        for k in range(KE + KH):
            nc.sync.dma_start(wg_sb[:, k * G4:(k + 1) * G4],
                              Wg[k * 128:(k + 1) * 128, :])

        prev_tok = None

        def emit_p2(m0):
            # one phase-2 output tile: rows m0..m0+w of [ROWS, VS] logits
            w = min(128, ROWS - m0)
            hh = wp.tile([128, KH * 128], F32, tag="hh")
            for k in range(KH):
                nc.sync.dma_start(hh[:, k * 128:k * 128 + w],
                                  hTd[k * 128:(k + 1) * 128, m0:m0 + w])
            hr = wp.tile([128, KH * 128], F32R, tag="hr")
            nc.vector.tensor_copy(hr[:, :], hh[:, :])
            for n in range(NCH):
                wr = wp.tile([128, KH * NCHUNK], F32R, tag="wr")
                for k in range(KH):
                    nc.vector.tensor_copy(
                        wr[:, k * NCHUNK:(k + 1) * NCHUNK],
                        fcw_sb[:, k * VS + n * NCHUNK:k * VS + (n + 1) * NCHUNK])
                pf = psB.tile([128, NCHUNK], F32, tag="pf")
                for k in range(KH):
                    nc.tensor.matmul(
                        pf[:w, :], hr[:, k * 128:k * 128 + w],
                        wr[:, k * NCHUNK:(k + 1) * NCHUNK],
                        start=(k == 0), stop=(k == KH - 1))
                ob = wp.tile([128, NCHUNK], F32, tag="ob")
                nc.vector.scalar_tensor_tensor(
                    out=ob[:w, :], in0=pf[:w, :], scalar=1.0,
                    in1=fcb_sb[:w, n * NCHUNK:(n + 1) * NCHUNK],
                    op0=OP.mult, op1=OP.add)
                t0 = m0 // B
                ntb = w // B
                dst = out[:, t0:t0 + ntb, n * NCHUNK:(n + 1) * NCHUNK]
                nc.sync.dma_start(dst.rearrange("b t n -> t b n"), ob[:w, :])

        # ---------------- phase 1: recurrence ----------------
        for t in range(1, S_):
            # x = emb[tok_{t-1}]
            if t == 1 or use_tf[t - 1]:
                off_ap = caps_sb[:, t - 1:t]
            else:
                off_ap = prev_tok[:, :]
            x = wp.tile([B, E], F32, tag="x")
            nc.gpsimd.indirect_dma_start(
                out=x[:, :], out_offset=None, in_=emb[:, :],
                in_offset=bass.IndirectOffsetOnAxis(ap=off_ap, axis=0))

            xT = wp.tile([128, KE * B], F32, tag="xT")
            for k in range(KE):
                pt = psT.tile([128, B], F32, tag="pt")
                nc.tensor.transpose(out=pt[:, :], in_=x[:, k * 128:(k + 1) * 128],
                                    identity=id_sb[:B, :B])
                nc.vector.tensor_copy(xT[:, k * B:(k + 1) * B], pt[:, :])

            # gates = x @ W_ih.T + h @ W_hh.T   (PSUM accumulate, 512 chunks)
            pg = psA.tile([B, G4], F32, tag="pg")
            for k in range(KE + KH):
                lhs = (xT[:, k * B:(k + 1) * B] if k < KE
                       else win(k - KE, t - 1))
                for nn in range(G4 // 512):
                    sl = slice(nn * 512, (nn + 1) * 512)
                    nc.tensor.matmul(
                        pg[:, sl], lhs,
                        wg_sb[:, k * G4 + nn * 512:k * G4 + (nn + 1) * 512],
                        start=(k == 0), stop=(k == KE + KH - 1))

            gates = gp.tile([B, G4], F32, tag="gates")
            nc.vector.scalar_tensor_tensor(
                out=gates[:, :], in0=pg[:, :], scalar=1.0,
                in1=bg_sb[:, :],
                op0=OP.mult, op1=OP.add)

            # activations: sigmoid(z) = 0.5*tanh(z/2)+0.5 (the tanh table is
            # ~13x more accurate than the sigmoid table; the recurrence must
            # track the fp32 reference to ~1e-6 to preserve argmax decisions)
            tif = ep.tile([B, 2 * H], F32, tag="tif")
            nc.scalar.activation(tif[:, :], gates[:, 0:2 * H], AF.Tanh, scale=0.5)
            tg = ep.tile([B, H], F32, tag="tg")
            nc.scalar.activation(tg[:, :], gates[:, 2 * H:3 * H], AF.Tanh)
            to_ = ep.tile([B, H], F32, tag="to")
            nc.scalar.activation(to_[:, :], gates[:, 3 * H:4 * H], AF.Tanh, scale=0.5)
            nc.vector.tensor_scalar(out=tif[:, :], in0=tif[:, :], scalar1=0.5,
                                    scalar2=0.5, op0=OP.mult, op1=OP.add)
            nc.vector.tensor_scalar(out=to_[:, :], in0=to_[:, :], scalar1=0.5,
                                    scalar2=0.5, op0=OP.mult, op1=OP.add)
            m1 = ep.tile([B, H], F32, tag="m1")
            nc.vector.tensor_mul(m1[:, :], tif[:, H:2 * H], c_sb[:, :])
            m2 = ep.tile([B, H], F32, tag="m2")
            nc.vector.tensor_mul(m2[:, :], tif[:, 0:H], tg[:, :])
            nc.vector.tensor_add(c_sb[:, :], m1[:, :], m2[:, :])
            tc_ = ep.tile([B, H], F32, tag="tg")
            nc.scalar.activation(tc_[:, :], c_sb[:, :], AF.Tanh)
            h = ep.tile([B, H], F32, tag="h")
            nc.vector.tensor_mul(h[:, :], to_[:, :], tc_[:, :])

            for k in range(KH):
                pt = psT.tile([128, B], F32, tag="pt")
                nc.tensor.transpose(out=pt[:, :], in_=h[:, k * 128:(k + 1) * 128],
                                    identity=id_sb[:B, :B])
                nc.vector.tensor_copy(win(k, t), pt[:, :])
                nc.sync.dma_start(hTd[k * 128:(k + 1) * 128, (t - 1) * B:t * B],
                                  win(k, t))

            if t >= 2 and t % 2 == 0 and (t // 2 - 1) * 128 + 128 <= ROWS:
                emit_p2((t // 2 - 1) * 128)

            if t < S_ - 1 and not use_tf[t]:
                # fc over this core's vocab shard + local top-1
                cmax = sp.tile([B, 8 * NCH], F32, tag="cmax")
                cidxf = sp.tile([B, 8 * NCH], F32, tag="cidxf")
                for n in range(NCH):
                    pf = psB.tile([B, NCHUNK], F32, tag="pf")
                    for k in range(KH):
                        nc.tensor.matmul(
                            pf[:, :], win(k, t),
                            fcw_sb[:, k * VS + n * NCHUNK:k * VS + (n + 1) * NCHUNK],
                            start=(k == 0), stop=(k == KH - 1))
                    lg = wp.tile([B, NCHUNK], F32, tag="lg")
                    nc.vector.scalar_tensor_tensor(
                        out=lg[:, :], in0=pf[:, :], scalar=1.0,
                        in1=fcb_sb[:B, n * NCHUNK:(n + 1) * NCHUNK],
                        op0=OP.mult, op1=OP.add)
                    nc.vector.max(out=cmax[:, 8 * n:8 * n + 8], in_=lg[:, :])
                    ci = sp.tile([B, 8], U32, tag="ci")
                    nc.vector.max_index(out=ci[:, :], in_max=cmax[:, 8 * n:8 * n + 8],
                                        in_values=lg[:, :])
                    nc.vector.tensor_copy(cidxf[:, 8 * n:8 * n + 8], ci[:, :])

                gmax = sp.tile([B, 8], F32, tag="gmax")
                nc.vector.max(out=gmax[:, :], in_=cmax[:, :])
                gpos = sp.tile([B, 8], U32, tag="gpos")
                nc.vector.max_index(out=gpos[:, :], in_max=gmax[:, :],
                                    in_values=cmax[:, :])
                gposf = sp.tile([B, 1], F32, tag="gposf")
                nc.vector.tensor_copy(gposf[:, :], gpos[:, 0:1])
                oh = sp.tile([B, 8 * NCH], F32, tag="oh")
                nc.vector.tensor_tensor(
                    oh[:, :], a64_sb[:, :],
                    gposf[:, 0:1].to_broadcast([B, 8 * NCH]), OP.is_equal)
                li = sp.tile([B, 8 * NCH], F32, tag="li")
                nc.vector.scalar_tensor_tensor(
                    out=li[:, :], in0=cidxf[:, :], scalar=1.0,
                    in1=cb_sb[:, :],
                    op0=OP.mult, op1=OP.add)
                sel = sp.tile([B, 8 * NCH], F32, tag="sel")
                nc.vector.tensor_mul(sel[:, :], oh[:, :], li[:, :])
                lidx = sp.tile([B, 1], F32, tag="lidx")
                nc.vector.tensor_reduce(out=lidx[:, :], in_=sel[:, :],
                                        axis=AX.X, op=OP.add)

                pk = sp.tile([B, 2], F32, tag="pk")
                nc.vector.tensor_copy(pk[:, 0:1], gmax[:, 0:1])
                nc.vector.scalar_tensor_tensor(
                    out=pk[:, 1:2], in0=lidx[:, :], scalar=1.0,
                    in1=co_sb[:, :],
                    op0=OP.mult, op1=OP.add)

                ptk = psT.tile([2, B], F32, tag="pt")
                nc.tensor.transpose(out=ptk[:, :], in_=pk[:, :],
                                    identity=id_sb[:B, :B])
                snd = sp.tile([2, B], F32, tag="snd")
                nc.vector.tensor_copy(snd[:, :], ptk[:, :])

                bin_ = dp.tile([2, B], F32, tag="bin")
                bout = dp.tile([2 * NCORES, B], F32, tag="bout")
                nc.sync.dma_start(bin_[:, :], snd[:, :])
                nc.gpsimd.collective_compute(
                    "AllGather", OP.bypass,
                    replica_groups=[list(range(NCORES))],
                    ins=[bin_.opt()], outs=[bout.opt()])
                rcv = sp.tile([2 * NCORES, B], F32, tag="rcv")
                nc.sync.dma_start(rcv[:, :], bout[:, :])

                pr = psT.tile([B, 2 * NCORES], F32, tag="pt")
                nc.tensor.transpose(out=pr[:, :], in_=rcv[:, :],
                                    identity=id_sb[:2 * NCORES, :2 * NCORES])
                comb = sp.tile([B, 2 * NCORES], F32, tag="comb")
                nc.vector.tensor_copy(comb[:, :], pr[:, :])
                combr = comb[:, :].rearrange("b (c k) -> b c k", k=2)
                v8 = sp.tile([B, NCORES], F32, tag="v8")
                nc.vector.tensor_copy(v8[:, :], combr[:, :, 0:1])
                i8 = sp.tile([B, NCORES], F32, tag="i8")
                nc.vector.tensor_copy(i8[:, :], combr[:, :, 1:2])

                gm2 = sp.tile([B, 8], F32, tag="gm2")
                nc.vector.max(out=gm2[:, :], in_=v8[:, :])
                gp2 = sp.tile([B, 8], U32, tag="gp2")
                nc.vector.max_index(out=gp2[:, :], in_max=gm2[:, :], in_values=v8[:, :])
                gp2f = sp.tile([B, 1], F32, tag="gp2f")
                nc.vector.tensor_copy(gp2f[:, :], gp2[:, 0:1])
                oh8 = sp.tile([B, 8], F32, tag="oh8")
                nc.vector.tensor_tensor(
                    oh8[:, :], a8_sb[:, :],
                    gp2f[:, 0:1].to_broadcast([B, 8]), OP.is_equal)
                s8 = sp.tile([B, 8], F32, tag="s8")
                nc.vector.tensor_mul(s8[:, :], oh8[:, :], i8[:, :])
                tkf = sp.tile([B, 1], F32, tag="tkf")
                nc.vector.tensor_reduce(out=tkf[:, :], in_=s8[:, :], axis=AX.X,
                                        op=OP.add)
                tki = sp.tile([B, 1], I32, tag="tki")
                nc.vector.tensor_copy(tki[:, :], tkf[:, :])
                prev_tok = tki
                nc.vector.tensor_copy(tokdbg[:, t - 1:t], tki[:, :])
            elif t < S_ - 1:
                nc.vector.tensor_copy(tokdbg[:, t - 1:t], caps_sb[:, t:t + 1])

        nc.sync.dma_start(toks[:, :], tokdbg[:, :])
        n_done = max(0, (S_ - 1) // 2 * 1)
        emitted = [(tt // 2 - 1) * 128 for tt in range(2, S_)
                   if tt % 2 == 0 and (tt // 2 - 1) * 128 + 128 <= ROWS]
        m0 = (emitted[-1] + 128) if emitted else 0
        while m0 < ROWS:
            emit_p2(m0)
            m0 += 128
        wgp_cm.__exit__(None, None, None)

    nc.compile()
    return nc


def kernel(**inputs):
    import warnings
    warnings.filterwarnings("ignore")
    from concourse.bass_utils import run_bass_kernel_spmd

    enc = np.asarray(inputs["encoder_hidden"], dtype=np.float32)
    caps = np.asarray(inputs["captions"], dtype=np.int32)
    use_tf = np.asarray(inputs["use_tf"]).astype(bool)
    emb = np.array(inputs["emb_table"], dtype=np.float32)
    emb[0] = 0.0
    W_ih = np.asarray(inputs["W_ih"], dtype=np.float32)
    W_hh = np.asarray(inputs["W_hh"], dtype=np.float32)
    b = (np.asarray(inputs["b_ih"], dtype=np.float32)
         + np.asarray(inputs["b_hh"], dtype=np.float32))
    fc_w = np.asarray(inputs["fc_w"], dtype=np.float32)
    fc_b = np.asarray(inputs["fc_b"], dtype=np.float32)

    S_ = int(os.environ.get("DEC_DEV_STEPS", caps.shape[1]))
    caps_u = np.ascontiguousarray(caps[:, :S_])
    use_tf_u = use_tf[:S_]

    nc = _build_nc([bool(v) for v in use_tf_u])

    Wg_np = np.ascontiguousarray(
        np.concatenate([W_ih.T, W_hh.T], axis=0), dtype=np.float32)
    common = {
        "encT": np.ascontiguousarray(enc.T),
        "caps": caps_u,
        "Wg": Wg_np,
        "bg": b[None, :].copy(),
        "emb": emb,
        "ident": np.eye(128, dtype=np.float32),
        "cbase": np.array([[(j // 8) * NCHUNK for j in range(8 * NCH)]],
                          dtype=np.float32),
        "ar64": np.arange(8 * NCH, dtype=np.float32)[None, :].copy(),
        "ar8": np.arange(8, dtype=np.float32)[None, :].copy(),
    }
    in_maps = []
    for c in range(NCORES):
        m = dict(common)
        m["fcw"] = np.ascontiguousarray(fc_w[c * VS:(c + 1) * VS].T)
        m["fcb"] = np.ascontiguousarray(fc_b[None, c * VS:(c + 1) * VS])
        m["coreoff"] = np.array([[float(c * VS)]], dtype=np.float32)
        in_maps.append(m)

    res = run_bass_kernel_spmd(nc, in_maps, core_ids=list(range(NCORES)))

    full = np.zeros((B, S_, V), dtype=np.float32)
    for c in range(NCORES):
        full[:, 1:, c * VS:(c + 1) * VS] = res.results[c]["out"]
    kernel.last_results = res
    return full


# revision 13
# speedup vs baseline: 1.1371x; 1.1371x over previous
"""Trainium2 Bass kernel for an LSTM caption decoder (teacher-forced).

Strategy (8 NeuronCores, SPMD):
  - The LSTM recurrence (63 sequential steps) is replicated on all cores in
    fp32; the big fc projection [H=512 -> V=32000] is vocab-sharded (4000
    per core).
  - On steps whose argmax feeds the next token (use_tf[t] == False), each
    core computes its local top-1 over its vocab shard, cores AllGather the
    (value, index) pairs, and every core selects the global argmax token.
  - The full logits output [B, S-1, V] is recomputed after the recurrence as
    one large batched matmul in float32r (reduced-precision fp32, 4x faster,
    ~1.4e-4 rel err), vocab-sharded, and DMA'd out.
  - The teacher-forcing pattern (use_tf) is baked into the generated program
    at build time; captions/weights remain runtime data.
"""

import os
import numpy as np

V, E, H = 32000, 256, 512
B, S = 64, 64
NCORES = 8
VS = V // NCORES          # 4000 vocab per core
NCHUNK = 500              # fc free-dim chunk (one PSUM bank)
NCH = VS // NCHUNK        # 8 chunks


def _build_nc(use_tf):
    import concourse.bass as bass
    import concourse.bacc as bacc
    import concourse.mybir as mybir
    from concourse.tile import TileContext
    from contextlib import ExitStack

    F32 = mybir.dt.float32
    F32R = mybir.dt.float32r
    I32 = mybir.dt.int32
    U32 = mybir.dt.uint32
    AF = mybir.ActivationFunctionType
    OP = mybir.AluOpType
    AX = mybir.AxisListType

    COLTILE = os.environ.get("DEC_COLTILE", "1") == "1"
    S_ = len(use_tf)
    NT = S_ - 1               # number of recurrence steps (t = 1..S_-1)
    KH = H // 128             # 4 k-tiles over hidden
    KE = E // 128             # 2 k-tiles over embed
    ROWS = NT * B             # stored h rows (h_1..h_{NT})
    G4 = 4 * H                # 2048 gate width

    nc = bacc.Bacc("TRN2", target_bir_lowering=False, debug=False,
                   num_devices=NCORES)

    encT = nc.dram_tensor("encT", [H, B], F32, kind="ExternalInput")
    caps = nc.dram_tensor("caps", [B, S_], I32, kind="ExternalInput")
    Wg = nc.dram_tensor("Wg", [E + H, G4], F32, kind="ExternalInput")
    bg = nc.dram_tensor("bg", [1, G4], F32, kind="ExternalInput")
    fcw = nc.dram_tensor("fcw", [H, VS], F32, kind="ExternalInput")
    fcb = nc.dram_tensor("fcb", [1, VS], F32, kind="ExternalInput")
    fcb2 = nc.dram_tensor("fcb2", [2, VS // 2], F32, kind="ExternalInput")
    emb = nc.dram_tensor("emb", [V, E], F32, kind="ExternalInput")
    coreoff = nc.dram_tensor("coreoff", [1, 1], F32, kind="ExternalInput")
    ident = nc.dram_tensor("ident", [128, 128], F32, kind="ExternalInput")
    cbase = nc.dram_tensor("cbase", [1, 8 * NCH], F32, kind="ExternalInput")
    ar64 = nc.dram_tensor("ar64", [1, 8 * NCH], F32, kind="ExternalInput")
    ar8 = nc.dram_tensor("ar8", [1, 8], F32, kind="ExternalInput")

    out = nc.dram_tensor("out", [B, NT, VS], F32, kind="ExternalOutput")
    toks = nc.dram_tensor("toks", [B, NT], I32, kind="ExternalOutput")
    hTd = nc.dram_tensor("hTd", [H, ROWS], F32)   # internal: transposed h's

    with TileContext(nc) as tc, ExitStack() as es:
        cp = es.enter_context(tc.tile_pool(name="const", bufs=1))
        dp = es.enter_context(tc.tile_pool(name="dram", bufs=2, space="DRAM"))
        gp = es.enter_context(tc.tile_pool(name="gates", bufs=1))
        ep = es.enter_context(tc.tile_pool(name="elem", bufs=1))
        wp = es.enter_context(tc.tile_pool(name="work", bufs=2))
        sp = es.enter_context(tc.tile_pool(name="small", bufs=2))
        psA = es.enter_context(tc.tile_pool(name="psA", bufs=1, space="PSUM"))
        psB = es.enter_context(tc.tile_pool(name="psB", bufs=3, space="PSUM"))
        psT = es.enter_context(tc.tile_pool(name="psT", bufs=1, space="PSUM"))

        # ---------------- persistent tiles ----------------
        hT_win = cp.tile([128, KH * 2 * B], F32)   # rolling 2-step window of h^T
        fcw_sb = cp.tile([128, KH * VS], F32)
        bg_sb = cp.tile([B, G4], F32)
        fcb2_sb = cp.tile([128, VS // 2], F32)
        id_sb = cp.tile([128, 128], F32)
        caps_sb = cp.tile([B, S_], I32)
        co_sb = cp.tile([B, 1], F32)
        cb_sb = cp.tile([B, 8 * NCH], F32)
        a64_sb = cp.tile([B, 8 * NCH], F32)
        a8_sb = cp.tile([B, 8], F32)
        c_sb = cp.tile([B, H], F32)
        tokdbg = cp.tile([B, NT], I32)

        def win(k, t):  # hT window column slice for step t, k-tile k
            base = k * 2 * B + (t % 2) * B
            return hT_win[:, base:base + B]

        for k in range(KH):
            nc.sync.dma_start(win(k, 0), encT[k * 128:(k + 1) * 128, :])
            nc.sync.dma_start(fcw_sb[:, k * VS:(k + 1) * VS],
                              fcw[k * 128:(k + 1) * 128, :])
        nc.sync.dma_start(bg_sb[:, :], bg[:, :].to_broadcast((B, G4)))
        nc.sync.dma_start(fcb2_sb[0:64, :], fcb2[0:1, :].to_broadcast((64, VS // 2)))
        nc.sync.dma_start(fcb2_sb[64:128, :], fcb2[1:2, :].to_broadcast((64, VS // 2)))
        nc.sync.dma_start(id_sb[:, :], ident[:, :])
        nc.sync.dma_start(caps_sb[:, :], caps[:, :])
        nc.sync.dma_start(co_sb[:, :], coreoff[:, :].to_broadcast((B, 1)))
        nc.sync.dma_start(cb_sb[:, :], cbase[:, :].to_broadcast((B, 8 * NCH)))
        nc.sync.dma_start(a64_sb[:, :], ar64[:, :].to_broadcast((B, 8 * NCH)))
        nc.sync.dma_start(a8_sb[:, :], ar8[:, :].to_broadcast((B, 8)))
        nc.vector.memset(c_sb[:, :], 0.0)
        nc.vector.memset(tokdbg[:, :], 0)

        wgp_cm = tc.tile_pool(name="wg", bufs=1)
        wgp = wgp_cm.__enter__()
        wg_sb = wgp.tile([128, (KE + KH) * G4], F32)
        for k in range(KE + KH):
            nc.sync.dma_start(wg_sb[:, k * G4:(k + 1) * G4],
                              Wg[k * 128:(k + 1) * 128, :])

        prev_tok = None
        next_m0 = [0]

        def emit_gather(off_ap):
            xt_ = wp.tile([B, E], F32, tag="x")
            nc.gpsimd.indirect_dma_start(
                out=xt_[:, :], out_offset=None, in_=emb[:, :],
                in_offset=bass.IndirectOffsetOnAxis(ap=off_ap, axis=0))
            return xt_

        def transpose_x(x_):
            xT_ = wp.tile([128, KE * B], F32, tag="xT")
            for k in range(KE):
                pt = psT.tile([128, B], F32, tag="pt")
                nc.tensor.transpose(out=pt[:, :], in_=x_[:, k * 128:(k + 1) * 128],
                                    identity=id_sb[:B, :B])
                nc.vector.tensor_copy(xT_[:, k * B:(k + 1) * B], pt[:, :])
            return xT_

        def emit_h_part(t_next):
            # h_{t_next-1} @ W_hh into a fresh gates psum tile (start=True)
            pgn = psA.tile([B, G4], F32, tag="pg")
            for k in range(KH):
                lhs = win(k, t_next - 1)
                for nn in range(4):
                    nc.tensor.matmul(
                        pgn[:, nn * 512:(nn + 1) * 512], lhs,
                        wg_sb[:, (KE + k) * G4 + nn * 512:(KE + k) * G4 + (nn + 1) * 512],
                        start=(k == 0), stop=False)
            return pgn

        def emit_x_part(pgn, xT_):
            for k in range(KE):
                for nn in range(4):
                    nc.tensor.matmul(
                        pgn[:, nn * 512:(nn + 1) * 512], xT_[:, k * B:(k + 1) * B],
                        wg_sb[:, k * G4 + nn * 512:k * G4 + (nn + 1) * 512],
                        start=False, stop=(k == KE - 1))

        def emit_p2(m0):
            # one phase-2 output tile: rows m0..m0+w of [ROWS, VS] logits
            w = min(128, ROWS - m0)
            hh = wp.tile([128, KH * 128], F32, tag="hh")
            for k in range(KH):
                nc.sync.dma_start(hh[:, k * 128:k * 128 + w],
                                  hTd[k * 128:(k + 1) * 128, m0:m0 + w])
            hr = wp.tile([128, KH * 128], F32R, tag="hr")
            nc.vector.tensor_copy(hr[:, :], hh[:, :])
            for n in range(NCH):
                wr = wp.tile([128, KH * NCHUNK], F32R, tag="wr")
                for k in range(KH):
                    nc.vector.tensor_copy(
                        wr[:, k * NCHUNK:(k + 1) * NCHUNK],
                        fcw_sb[:, k * VS + n * NCHUNK:k * VS + (n + 1) * NCHUNK])
                pf = psB.tile([128, NCHUNK], F32, tag="pf")
                for k in range(KH):
                    nc.tensor.matmul(
                        pf[:w, :], hr[:, k * 128:k * 128 + w],
                        wr[:, k * NCHUNK:(k + 1) * NCHUNK],
                        start=(k == 0), stop=(k == KH - 1))
                fb = wp.tile([128, NCHUNK], F32, tag="fb")
                nc.sync.dma_start(
                    fb[:, :],
                    fcb[0:1, n * NCHUNK:(n + 1) * NCHUNK].to_broadcast((128, NCHUNK)))
                ob = wp.tile([128, NCHUNK], F32, tag="ob")
                nc.vector.scalar_tensor_tensor(
                    out=ob[:w, :], in0=pf[:w, :], scalar=1.0,
                    in1=fb[:w, :], op0=OP.mult, op1=OP.add)
                t0 = m0 // B
                ntb = w // B
                dst = out[:, t0:t0 + ntb, n * NCHUNK:(n + 1) * NCHUNK]
                nc.sync.dma_start(dst.rearrange("b t n -> t b n"), ob[:w, :])

        def drain_p2(t_done, budget):
            while budget > 0:
                m0 = next_m0[0]
                if m0 >= ROWS:
                    return
                w = min(128, ROWS - m0)
                if m0 + w > t_done * B:
                    return
                emit_p2(m0)
                next_m0[0] += 128
                budget -= 1

        # prologue: gates for step 1 (token = captions[:, 0])
        pg_cur = emit_h_part(1)
        emit_x_part(pg_cur, transpose_x(emit_gather(caps_sb[:, 0:1])))

        # ---------------- phase 1: recurrence ----------------
        for t in range(1, S_):
            gates = gp.tile([B, G4], F32, tag="gates")
            nc.vector.scalar_tensor_tensor(
                out=gates[:, :], in0=pg_cur[:, :], scalar=1.0,
                in1=bg_sb[:, :], op0=OP.mult, op1=OP.add)

            # sigmoid(z) = 0.5*tanh(z/2)+0.5 (tanh table is ~13x more accurate
            # than the sigmoid table; recurrence must track fp32 to ~1e-6)
            tif = ep.tile([B, 2 * H], F32, tag="tif")
            nc.scalar.activation(tif[:, :], gates[:, 0:2 * H], AF.Tanh, scale=0.5)
            tg = ep.tile([B, H], F32, tag="tg")
            nc.scalar.activation(tg[:, :], gates[:, 2 * H:3 * H], AF.Tanh)
            to_ = ep.tile([B, H], F32, tag="to")
            nc.scalar.activation(to_[:, :], gates[:, 3 * H:4 * H], AF.Tanh, scale=0.5)
            nc.vector.tensor_scalar(out=tif[:, :], in0=tif[:, :], scalar1=0.5,
                                    scalar2=0.5, op0=OP.mult, op1=OP.add)
            nc.vector.tensor_scalar(out=to_[:, :], in0=to_[:, :], scalar1=0.5,
                                    scalar2=0.5, op0=OP.mult, op1=OP.add)
            m1 = ep.tile([B, H], F32, tag="m1")
            nc.vector.tensor_mul(m1[:, :], tif[:, H:2 * H], c_sb[:, :])
            m2 = ep.tile([B, H], F32, tag="m2")
            nc.vector.tensor_mul(m2[:, :], tif[:, 0:H], tg[:, :])
            nc.vector.tensor_add(c_sb[:, :], m1[:, :], m2[:, :])
            tc_ = ep.tile([B, H], F32, tag="tg")
            nc.scalar.activation(tc_[:, :], c_sb[:, :], AF.Tanh)
            h = ep.tile([B, H], F32, tag="h")
            nc.vector.tensor_mul(h[:, :], to_[:, :], tc_[:, :])

            for k in range(KH):
                pt = psT.tile([128, B], F32, tag="pt")
                nc.tensor.transpose(out=pt[:, :], in_=h[:, k * 128:(k + 1) * 128],
                                    identity=id_sb[:B, :B])
                nc.vector.tensor_copy(win(k, t), pt[:, :])
                nc.sync.dma_start(hTd[k * 128:(k + 1) * 128, (t - 1) * B:t * B],
                                  win(k, t))

            if t < S_ - 1 and not use_tf[t]:
                # fc over this core's vocab shard, col-tiled (chunk pairs n, n+4
                # run concurrently in PE column groups 0/1) + local top-1
                cmAB = sp.tile([128, 4 * 8], F32, tag="cmAB")
                ciAB = sp.tile([128, 4 * 8], F32, tag="ciAB")
                for n in range(NCH // 2):
                    pf = psB.tile([128, NCHUNK], F32, tag="pf")
                    for k in range(KH):
                        kwA = dict(tile_position=(0, 0)) if COLTILE else {}
                        kwB = dict(tile_position=(0, 64)) if COLTILE else {}
                        nc.tensor.matmul(
                            pf[0:64, :], win(k, t),
                            fcw_sb[:, k * VS + n * NCHUNK:k * VS + (n + 1) * NCHUNK],
                            start=(k == 0), stop=(k == KH - 1), **kwA)
                        nc.tensor.matmul(
                            pf[64:128, :], win(k, t),
                            fcw_sb[:, k * VS + (n + 4) * NCHUNK:k * VS + (n + 5) * NCHUNK],
                            start=(k == 0), stop=(k == KH - 1), **kwB)
                    lg = wp.tile([128, NCHUNK], F32, tag="lg")
                    nc.vector.scalar_tensor_tensor(
                        out=lg[:, :], in0=pf[:, :], scalar=1.0,
                        in1=fcb2_sb[:, n * NCHUNK:(n + 1) * NCHUNK],
                        op0=OP.mult, op1=OP.add)
                    nc.vector.max(out=cmAB[:, 8 * n:8 * n + 8], in_=lg[:, :])
                    ci = sp.tile([128, 8], U32, tag="ci")
                    nc.vector.max_index(out=ci[:, :], in_max=cmAB[:, 8 * n:8 * n + 8],
                                        in_values=lg[:, :])
                    nc.vector.tensor_copy(ciAB[:, 8 * n:8 * n + 8], ci[:, :])

                # gather both halves onto partitions 0-63
                cm = sp.tile([B, 8 * NCH], F32, tag="cm")
                ci_f = sp.tile([B, 8 * NCH], F32, tag="cif")
                nc.vector.tensor_copy(cm[:, 0:32], cmAB[0:64, :])
                nc.vector.tensor_copy(ci_f[:, 0:32], ciAB[0:64, :])
                nc.sync.dma_start(cm[:, 32:64], cmAB[64:128, :])
                nc.sync.dma_start(ci_f[:, 32:64], ciAB[64:128, :])

                gmax = sp.tile([B, 8], F32, tag="gmax")
                nc.vector.max(out=gmax[:, :], in_=cm[:, :])
                gpos = sp.tile([B, 8], U32, tag="gpos")
                nc.vector.max_index(out=gpos[:, :], in_max=gmax[:, :],
                                    in_values=cm[:, :])
                gposf = sp.tile([B, 1], F32, tag="gposf")
                nc.vector.tensor_copy(gposf[:, :], gpos[:, 0:1])
                oh = sp.tile([B, 8 * NCH], F32, tag="oh")
                nc.vector.tensor_tensor(
                    oh[:, :], a64_sb[:, :],
                    gposf[:, 0:1].to_broadcast([B, 8 * NCH]), OP.is_equal)
                li = sp.tile([B, 8 * NCH], F32, tag="li")
                nc.vector.scalar_tensor_tensor(
                    out=li[:, :], in0=ci_f[:, :], scalar=1.0,
                    in1=cb_sb[:, :], op0=OP.mult, op1=OP.add)
                sel = sp.tile([B, 8 * NCH], F32, tag="sel")
                nc.vector.tensor_mul(sel[:, :], oh[:, :], li[:, :])
                lidx = sp.tile([B, 1], F32, tag="lidx")
                nc.vector.tensor_reduce(out=lidx[:, :], in_=sel[:, :],
                                        axis=AX.X, op=OP.add)

                pk = sp.tile([B, 2], F32, tag="pk")
                nc.vector.tensor_copy(pk[:, 0:1], gmax[:, 0:1])
                nc.vector.scalar_tensor_tensor(
                    out=pk[:, 1:2], in0=lidx[:, :], scalar=1.0,
                    in1=co_sb[:, :], op0=OP.mult, op1=OP.add)

                ptk = psT.tile([2, B], F32, tag="pt")
                nc.tensor.transpose(out=ptk[:, :], in_=pk[:, :],
                                    identity=id_sb[:B, :B])
                snd = sp.tile([2, B], F32, tag="snd")
                nc.vector.tensor_copy(snd[:, :], ptk[:, :])

                bin_ = dp.tile([2, B], F32, tag="bin")
                bout = dp.tile([2 * NCORES, B], F32, tag="bout")
                nc.sync.dma_start(bin_[:, :], snd[:, :])
                nc.gpsimd.collective_compute(
                    "AllGather", OP.bypass,
                    replica_groups=[list(range(NCORES))],
                    ins=[bin_.opt()], outs=[bout.opt()])

                # fill the collective window with useful PE work
                if t + 1 < S_:
                    pg_cur = emit_h_part(t + 1)
                drain_p2(t, budget=2)

                rcv = sp.tile([2 * NCORES, B], F32, tag="rcv")
                nc.sync.dma_start(rcv[:, :], bout[:, :])
                pr = psT.tile([B, 2 * NCORES], F32, tag="pt")
                nc.tensor.transpose(out=pr[:, :], in_=rcv[:, :],
                                    identity=id_sb[:2 * NCORES, :2 * NCORES])
                comb = sp.tile([B, 2 * NCORES], F32, tag="comb")
                nc.vector.tensor_copy(comb[:, :], pr[:, :])
                combr = comb[:, :].rearrange("b (c k) -> b c k", k=2)
                v8 = sp.tile([B, NCORES], F32, tag="v8")
                nc.vector.tensor_copy(v8[:, :], combr[:, :, 0:1])
                i8 = sp.tile([B, NCORES], F32, tag="i8")
                nc.vector.tensor_copy(i8[:, :], combr[:, :, 1:2])

                gm2 = sp.tile([B, 8], F32, tag="gm2")
                nc.vector.max(out=gm2[:, :], in_=v8[:, :])
                gp2 = sp.tile([B, 8], U32, tag="gp2")
                nc.vector.max_index(out=gp2[:, :], in_max=gm2[:, :], in_values=v8[:, :])
                gp2f = sp.tile([B, 1], F32, tag="gp2f")
                nc.vector.tensor_copy(gp2f[:, :], gp2[:, 0:1])
                oh8 = sp.tile([B, 8], F32, tag="oh8")
                nc.vector.tensor_tensor(
                    oh8[:, :], a8_sb[:, :],
                    gp2f[:, 0:1].to_broadcast([B, 8]), OP.is_equal)
                s8 = sp.tile([B, 8], F32, tag="s8")
                nc.vector.tensor_mul(s8[:, :], oh8[:, :], i8[:, :])
                tkf = sp.tile([B, 1], F32, tag="tkf")
                nc.vector.tensor_reduce(out=tkf[:, :], in_=s8[:, :], axis=AX.X,
                                        op=OP.add)
                tki = sp.tile([B, 1], I32, tag="tki")
                nc.vector.tensor_copy(tki[:, :], tkf[:, :])
                prev_tok = tki
                nc.vector.tensor_copy(tokdbg[:, t - 1:t], tki[:, :])

                if t + 1 < S_:
                    emit_x_part(pg_cur, transpose_x(emit_gather(prev_tok[:, :])))
            elif t < S_ - 1:
                nc.vector.tensor_copy(tokdbg[:, t - 1:t], caps_sb[:, t:t + 1])
                pg_cur = emit_h_part(t + 1)
                emit_x_part(pg_cur, transpose_x(emit_gather(caps_sb[:, t:t + 1])))
                drain_p2(t, budget=1)
            elif t == S_ - 1:
                pass

        drain_p2(NT, budget=1000)
        nc.sync.dma_start(toks[:, :], tokdbg[:, :])
        wgp_cm.__exit__(None, None, None)

    nc.compile()
    return nc


def kernel(**inputs):
    import warnings
    warnings.filterwarnings("ignore")
    from concourse.bass_utils import run_bass_kernel_spmd

    enc = np.asarray(inputs["encoder_hidden"], dtype=np.float32)
    caps = np.asarray(inputs["captions"], dtype=np.int32)
    use_tf = np.asarray(inputs["use_tf"]).astype(bool)
    emb = np.array(inputs["emb_table"], dtype=np.float32)
    emb[0] = 0.0
    W_ih = np.asarray(inputs["W_ih"], dtype=np.float32)
    W_hh = np.asarray(inputs["W_hh"], dtype=np.float32)
    b = (np.asarray(inputs["b_ih"], dtype=np.float32)
         + np.asarray(inputs["b_hh"], dtype=np.float32))
    fc_w = np.asarray(inputs["fc_w"], dtype=np.float32)
    fc_b = np.asarray(inputs["fc_b"], dtype=np.float32)

    S_ = int(os.environ.get("DEC_DEV_STEPS", caps.shape[1]))
    caps_u = np.ascontiguousarray(caps[:, :S_])
    use_tf_u = use_tf[:S_]

    nc = _build_nc([bool(v) for v in use_tf_u])

    Wg_np = np.ascontiguousarray(
        np.concatenate([W_ih.T, W_hh.T], axis=0), dtype=np.float32)
    common = {
        "encT": np.ascontiguousarray(enc.T),
        "caps": caps_u,
        "Wg": Wg_np,
        "bg": b[None, :].copy(),
        "emb": emb,
        "ident": np.eye(128, dtype=np.float32),
        "cbase": np.array([[((j // 8) if j < 32 else (j - 32) // 8 + 4) * NCHUNK
                            for j in range(8 * NCH)]], dtype=np.float32),
        "ar64": np.arange(8 * NCH, dtype=np.float32)[None, :].copy(),
        "ar8": np.arange(8, dtype=np.float32)[None, :].copy(),
    }
    in_maps = []
    for c in range(NCORES):
        m = dict(common)
        m["fcw"] = np.ascontiguousarray(fc_w[c * VS:(c + 1) * VS].T)
        fcb_c = fc_b[c * VS:(c + 1) * VS]
        m["fcb"] = np.ascontiguousarray(fcb_c[None, :])
        m["fcb2"] = np.ascontiguousarray(
            np.stack([fcb_c[:VS // 2], fcb_c[VS // 2:]], axis=0))
        m["coreoff"] = np.array([[float(c * VS)]], dtype=np.float32)
        in_maps.append(m)

    res = run_bass_kernel_spmd(nc, in_maps, core_ids=list(range(NCORES)))

    full = np.zeros((B, S_, V), dtype=np.float32)
    for c in range(NCORES):
        full[:, 1:, c * VS:(c + 1) * VS] = res.results[c]["out"]
    kernel.last_results = res
    return full
